# revision 1
# baseline (speedup 1.0000x reference)
"""SGConv (K=2) GNN message-passing kernel for Trainium2 (8 NeuronCores).

out = (D^{-1/2} (A+I) D^{-1/2})^2 @ x @ W.T

Strategy:
  - Project first: h0 = x @ W.T (propagation commutes with the linear map),
    so both sparse hops run on 64-dim features instead of 256-dim.
  - Shard nodes across 8 cores; partition edges by destination node.
  - AllGather the projected features so every core holds the full feature
    table in DRAM; gather source rows with the bulk InstDMAGatherAnt path.
    int16 index limit -> the table is split in two parts (A/B) at a tile
    boundary of each core's shard; each part is AllGathered separately so
    the second hop's gathers can start before the first hop fully drains.
  - Segment-sum per destination: edges are grouped per 128-dst output tile;
    per 128-edge chunk, DVE builds a one-hot matrix is_equal(iota, dstloc)
    (exact in bf16) and TensorE accumulates segmat.T @ (norm * gathered)
    into the tile's PSUM. Each output row is produced exactly once -> no
    scatter races. The norm scaling is folded into the gathered values and
    optionally down-cast (GNN_DT) to speed up the PE.

Self-contained: hardcodes only NCORES=8; all shapes derived from inputs.
"""

import os
import numpy as np

from concourse import bacc, mybir, tile
from concourse.bass_utils import run_bass_kernel_spmd

NCORES = 8
P = 128
F32 = mybir.dt.float32
I16 = mybir.dt.int16

# Chunks (of 128 gathered rows) per dma_gather instruction. HW limit: the
# per-engine SWDGE descriptor ring holds 128 descriptors and a gather needs
# num_idxs/16 + 1 of them; 1024 idxs (65 descs) is verified safe.
SEG_CHUNKS = int(os.environ.get("GNN_SEG", "8"))

# Matmul dtype for the segment-sum path: f32 | bf16 | f16.
DT_NAME = os.environ.get("GNN_DT", "f32")
DT = {"f32": mybir.dt.float32, "bf16": mybir.dt.bfloat16,
      "f16": mybir.dt.float16}[DT_NAME]

LAST_RESULTS = None  # BassKernelResults of the last run (for test harness)


def _ceil(a, b):
    return -(-a // b)


def _wrap_idx(idx):
    """int16 [n] -> dma_gather layout [128, n//16]: wrapped in 16 partitions
    (unwrapped[i] = buf[i % 16, i // 16]) and replicated across the 8 Q7
    core groups."""
    n = idx.shape[0]
    assert n % 16 == 0
    w = np.ascontiguousarray(idx.reshape(n // 16, 16).T).astype(np.int16)
    return np.ascontiguousarray(np.tile(w, (8, 1)))


def _prepare(x, edge_index, W):
    """Host-side sharding/layout prep. Returns (dims, nch_u, in_maps)."""
    x = np.ascontiguousarray(np.asarray(x, dtype=np.float32))
    W = np.ascontiguousarray(np.asarray(W, dtype=np.float32))
    ei = np.asarray(edge_index).astype(np.int64)

    N, Din = x.shape
    Dout = int(W.shape[0])
    assert N % NCORES == 0, (N, NCORES)
    PN = N // NCORES
    T = _ceil(PN, P)
    assert Din % P == 0
    KT = Din // P
    assert T >= 2, "need at least 2 tiles per core for the A/B table split"
    TS = T // 2              # tile index where part A ends
    RA = TS * P              # rows of part A per shard
    RB = PN - RA             # rows of part B per shard
    assert NCORES * RA < 2**15 and NCORES * RB < 2**15

    src = np.concatenate([ei[0], np.arange(N, dtype=np.int64)])
    dst = np.concatenate([ei[1], np.arange(N, dtype=np.int64)])
    deg = np.bincount(dst, minlength=N).astype(np.float64)
    dinv = 1.0 / np.sqrt(np.maximum(deg, 1e-12))
    norm = (dinv[src] * dinv[dst]).astype(np.float32)

    core_of = dst // PN
    tloc = dst % PN
    tile_of = tloc // P
    dstloc = (tloc % P).astype(np.float32)

    s_core = src // PN
    s_off = src % PN
    part_of = (s_off >= RA).astype(np.int64)
    srcloc = np.where(part_of == 1,
                      s_core * RB + (s_off - RA),
                      s_core * RA + s_off)

    key = (core_of * T + tile_of) * 2 + part_of
    order = np.argsort(key, kind="stable")
    s_srcloc = srcloc[order]
    s_dstloc = dstloc[order]
    s_norm = norm[order]

    cnt = np.bincount(key, minlength=NCORES * T * 2).reshape(NCORES, T, 2)
    nch = -(-cnt // P)  # chunks needed per (core, tile, part)
    nch_u = nch.max(axis=0)  # [T, 2] cross-core uniform schedule
    NL = int(nch_u[:, 0].sum())
    NH = int(nch_u[:, 1].sum())

    starts = np.zeros(NCORES * T * 2 + 1, np.int64)
    starts[1:] = np.cumsum(cnt.reshape(-1))

    iota = np.ascontiguousarray(
        np.tile(np.arange(P, dtype=np.float32), (P, 1)))
    wt = np.ascontiguousarray(
        W.T.reshape(KT, P, Dout).transpose(1, 0, 2).reshape(P, KT * Dout))

    in_maps = []
    for c in range(NCORES):
        idx_f = [np.zeros(NL * P, np.int64), np.zeros(NH * P, np.int64)]
        dl_f = [np.zeros(NL * P, np.float32), np.zeros(NH * P, np.float32)]
        nm_f = [np.zeros(NL * P, np.float32), np.zeros(NH * P, np.float32)]
        off = [0, 0]
        for t in range(T):
            for h in (0, 1):
                k = (c * T + t) * 2 + h
                a, b = int(starts[k]), int(starts[k + 1])
                n = b - a
                o = off[h] * P
                idx_f[h][o:o + n] = s_srcloc[a:b]
                dl_f[h][o:o + n] = s_dstloc[a:b]
                nm_f[h][o:o + n] = s_norm[a:b]
                off[h] += int(nch_u[t, h])
        assert off[0] == NL and off[1] == NH

        xs = x[c * PN:(c + 1) * PN]
        xt = np.ascontiguousarray(
            xs.T.reshape(KT, P, PN).transpose(1, 0, 2).reshape(P, KT * PN))

        in_maps.append({
            "xt": xt,
            "wt": wt,
            "iota": iota,
            "idxlo": _wrap_idx(idx_f[0].astype(np.int16)),
            "idxhi": _wrap_idx(idx_f[1].astype(np.int16)),
            "dllo": np.ascontiguousarray(dl_f[0].reshape(NL, P).T),
            "nmlo": np.ascontiguousarray(nm_f[0].reshape(NL, P).T),
            "dlhi": np.ascontiguousarray(dl_f[1].reshape(NH, P).T),
            "nmhi": np.ascontiguousarray(nm_f[1].reshape(NH, P).T),
        })

    dims = dict(N=N, PN=PN, T=T, KT=KT, Din=Din, Dout=Dout, TS=TS,
                RA=RA, RB=RB)
    return dims, nch_u, in_maps


def _build(dims, nch_u, ablate=()):
    N, PN, T, KT, Dout = (dims["N"], dims["PN"], dims["T"], dims["KT"],
                          dims["Dout"])
    TS, RA, RB = dims["TS"], dims["RA"], dims["RB"]
    NL = int(nch_u[:, 0].sum())
    NH = int(nch_u[:, 1].sum())

    nc = bacc.Bacc("TRN2", target_bir_lowering=False, debug=False,
                   num_devices=NCORES)

    xt_d = nc.dram_tensor("xt", [P, KT * PN], F32, kind="ExternalInput")
    wt_d = nc.dram_tensor("wt", [P, KT * Dout], F32, kind="ExternalInput")
    iota_d = nc.dram_tensor("iota", [P, P], F32, kind="ExternalInput")
    idxlo_d = nc.dram_tensor("idxlo", [P, NL * 8], I16, kind="ExternalInput")
    idxhi_d = nc.dram_tensor("idxhi", [P, NH * 8], I16, kind="ExternalInput")
    dllo_d = nc.dram_tensor("dllo", [P, NL], F32, kind="ExternalInput")
    nmlo_d = nc.dram_tensor("nmlo", [P, NL], F32, kind="ExternalInput")
    dlhi_d = nc.dram_tensor("dlhi", [P, NH], F32, kind="ExternalInput")
    nmhi_d = nc.dram_tensor("nmhi", [P, NH], F32, kind="ExternalInput")
    out_d = nc.dram_tensor("out", [PN, Dout], F32, kind="ExternalOutput")

    # per-part shard outputs and AllGathered tables
    h0sA = nc.dram_tensor("h0sA", [RA, Dout], F32)
    h0sB = nc.dram_tensor("h0sB", [RB, Dout], F32)
    h0fA = nc.dram_tensor("h0fA", [NCORES * RA, Dout], F32,
                          addr_space="Shared")
    h0fB = nc.dram_tensor("h0fB", [NCORES * RB, Dout], F32,
                          addr_space="Shared")
    h1sA = nc.dram_tensor("h1sA", [RA, Dout], F32)
    h1sB = nc.dram_tensor("h1sB", [RB, Dout], F32)
    h1fA = nc.dram_tensor("h1fA", [NCORES * RA, Dout], F32,
                          addr_space="Shared")
    h1fB = nc.dram_tensor("h1fB", [NCORES * RB, Dout], F32,
                          addr_space="Shared")

    rg = [list(range(NCORES))]

    def allgather(src, dst):
        if "noag" in ablate:
            nc.gpsimd.dma_start(out=dst[0:src.shape[0], :], in_=src[:, :])
        else:
            nc.gpsimd.collective_compute(
                "AllGather", mybir.AluOpType.bypass, replica_groups=rg,
                ins=[src.ap().opt()], outs=[dst.ap().opt()])

    with tile.TileContext(nc) as tc:
        with tc.tile_pool(name="const", bufs=1) as constp:
            wts = constp.tile([P, KT * Dout], F32)
            nc.sync.dma_start(out=wts[:], in_=wt_d[:, :])
            iota_t = constp.tile([P, P], F32)
            nc.sync.dma_start(out=iota_t[:], in_=iota_d[:, :])
            idxlo_t = constp.tile([P, NL * 8], I16)
            nc.sync.dma_start(out=idxlo_t[:], in_=idxlo_d[:, :])
            idxhi_t = constp.tile([P, NH * 8], I16)
            nc.sync.dma_start(out=idxhi_t[:], in_=idxhi_d[:, :])
            dllo_t = constp.tile([P, NL], F32)
            nc.sync.dma_start(out=dllo_t[:], in_=dllo_d[:, :])
            nmlo_t = constp.tile([P, NL], F32)
            nc.sync.dma_start(out=nmlo_t[:], in_=nmlo_d[:, :])
            dlhi_t = constp.tile([P, NH], F32)
            nc.sync.dma_start(out=dlhi_t[:], in_=dlhi_d[:, :])
            nmhi_t = constp.tile([P, NH], F32)
            nc.sync.dma_start(out=nmhi_t[:], in_=nmhi_d[:, :])

            # ---------------- projection: h0 = x @ W.T ----------------
            with tc.tile_pool(name="proj", bufs=1) as projp, \
                 tc.tile_pool(name="ppsum", bufs=4, space="PSUM") as ppsum, \
                 tc.tile_pool(name="pout", bufs=3) as poutp:
                xts = projp.tile([P, KT * PN], F32)
                nc.sync.dma_start(out=xts[:], in_=xt_d[:, :])
                for m in range(T):
                    mw = min(P, PN - m * P)
                    ps = ppsum.tile([P, Dout], F32)
                    for k in range(KT):
                        nc.tensor.matmul(
                            out=ps[:mw, :],
                            lhsT=xts[:, k * PN + m * P: k * PN + m * P + mw],
                            rhs=wts[:, k * Dout:(k + 1) * Dout],
                            start=(k == 0), stop=(k == KT - 1))
                    ht = poutp.tile([P, Dout], F32)
                    nc.scalar.copy(out=ht[:mw, :], in_=ps[:mw, :])
                    if m < TS:
                        nc.sync.dma_start(out=h0sA[m * P:m * P + mw, :],
                                          in_=ht[:mw, :])
                    else:
                        nc.sync.dma_start(
                            out=h0sB[m * P - RA:m * P - RA + mw, :],
                            in_=ht[:mw, :])
                    if m == TS - 1:
                        allgather(h0sA, h0fA)
                allgather(h0sB, h0fB)

            def hop(tblA, tblB, dst_write):
                with tc.tile_pool(name="vals", bufs=1) as valsp, \
                     tc.tile_pool(name="stage", bufs=4) as stagep, \
                     tc.tile_pool(name="seg", bufs=4) as segp, \
                     tc.tile_pool(name="hpsum", bufs=4, space="PSUM") as hps, \
                     tc.tile_pool(name="hout", bufs=3) as houtp:
                    vlo = valsp.tile([P, max(NL, 1) * Dout], DT, tag="vlo")
                    vhi = valsp.tile([P, max(NH, 1) * Dout], DT, tag="vhi")
                    seg_of = {}
                    for vt, nblk, idx_t, h in ((vlo, NL, idxlo_t, 0),
                                               (vhi, NH, idxhi_t, 1)):
                        tbl = tblA if h == 0 else tblB
                        dl_t = dllo_t if h == 0 else dlhi_t
                        nm_t = nmlo_t if h == 0 else nmhi_t
                        s0 = 0
                        while s0 < nblk:
                            s1 = min(s0 + SEG_CHUNKS, nblk)
                            nb = s1 - s0
                            if DT is F32:
                                stg = vt[:, s0 * Dout:s1 * Dout].rearrange(
                                    "p (b f) -> p b f", f=Dout)
                            else:
                                stg_t = stagep.tile(
                                    [P, SEG_CHUNKS * Dout], F32, tag="stg")
                                stg = stg_t[:, :nb * Dout].rearrange(
                                    "p (b f) -> p b f", f=Dout)
                            if "nogather" in ablate:
                                nc.vector.memset(stg, 0.25)
                            else:
                                nc.gpsimd.dma_gather(
                                    out_ap=stg,
                                    in_ap=tbl[:, :],
                                    idxs_ap=idx_t[:, s0 * 8:s1 * 8],
                                    num_idxs=nb * P,
                                    num_idxs_reg=nb * P,
                                    elem_size=Dout)
                            # fold norm into the gathered values (+ cast)
                            nc.vector.tensor_tensor(
                                out=vt[:, s0 * Dout:s1 * Dout].rearrange(
                                    "p (b f) -> p b f", f=Dout),
                                in0=stg,
                                in1=nm_t[:, s0:s1].unsqueeze(-1).broadcast_to(
                                    [P, nb, Dout]),
                                op=mybir.AluOpType.mult)
                            # batched one-hot build for these chunks
                            sg = segp.tile([P, SEG_CHUNKS * P], DT,
                                           tag=f"sg{h}")
                            nc.vector.tensor_tensor(
                                out=sg[:, :nb * P].rearrange(
                                    "p (b f) -> p b f", f=P),
                                in0=iota_t[:].unsqueeze(1).broadcast_to(
                                    [P, nb, P]),
                                in1=dl_t[:, s0:s1].unsqueeze(-1).broadcast_to(
                                    [P, nb, P]),
                                op=mybir.AluOpType.is_equal)
                            for i in range(nb):
                                seg_of[(h, s0 + i)] = (sg, i)
                            s0 = s1

                    ofs = [0, 0]
                    for t in range(T):
                        tw = min(P, PN - t * P)
                        nlo = int(nch_u[t, 0])
                        nhi = int(nch_u[t, 1])
                        chunks = ([(0, ofs[0] + i) for i in range(nlo)]
                                  + [(1, ofs[1] + i) for i in range(nhi)])
                        ofs[0] += nlo
                        ofs[1] += nhi
                        ps = hps.tile([P, Dout], F32)
                        for ci, (h, blk) in enumerate(chunks):
                            vt = vlo if h == 0 else vhi
                            sg, si = seg_of[(h, blk)]
                            nc.tensor.matmul(
                                out=ps[:, :],
                                lhsT=sg[:, si * P:(si + 1) * P],
                                rhs=vt[:, blk * Dout:(blk + 1) * Dout],
                                start=(ci == 0),
                                stop=(ci == len(chunks) - 1))
                        ot = houtp.tile([P, Dout], F32)
                        nc.scalar.copy(out=ot[:tw, :], in_=ps[:tw, :])
                        dst_write(t, tw, ot)

            def hop1_write(t, tw, ot):
                if t < TS:
                    nc.sync.dma_start(out=h1sA[t * P:t * P + tw, :],
                                      in_=ot[:tw, :])
                    if t == TS - 1:
                        allgather(h1sA, h1fA)
                else:
                    nc.sync.dma_start(
                        out=h1sB[t * P - RA:t * P - RA + tw, :],
                        in_=ot[:tw, :])

            def out_write(t, tw, ot):
                nc.sync.dma_start(out=out_d[t * P:t * P + tw, :],
                                  in_=ot[:tw, :])

            if "nohop" in ablate:
                nc.gpsimd.dma_start(out=out_d[0:RA, :], in_=h0sA[:, :])
                nc.gpsimd.dma_start(out=out_d[RA:PN, :], in_=h0sB[:, :])
            elif "nohop2" in ablate:
                hop(h0fA, h0fB, out_write)
            else:
                hop(h0fA, h0fB, hop1_write)
                allgather(h1sB, h1fB)
                hop(h1fA, h1fB, out_write)

    nc.compile()
    return nc


def kernel(**inputs):
    global LAST_RESULTS
    x = inputs["x"]
    W = inputs["W"]
    edge_index = inputs["edge_index"]

    dims, nch_u, in_maps = _prepare(x, edge_index, W)
    ablate = tuple(a for a in os.environ.get("GNN_ABLATE", "").split(",") if a)
    nc = _build(dims, nch_u, ablate=ablate)

    trace = bool(int(os.environ.get("GNN_TRACE", "0")))
    kwargs = {}
    if trace:
        kwargs["trace"] = True
        kwargs["trace_cores"] = list(range(NCORES))
    res = run_bass_kernel_spmd(nc, in_maps, core_ids=list(range(NCORES)),
                               **kwargs)
    LAST_RESULTS = res
    out = np.concatenate(
        [res.results[c]["out"] for c in range(NCORES)], axis=0)
    return np.ascontiguousarray(out, dtype=np.float32)

